# revision 1
# baseline (speedup 1.0000x reference)
"""Trainium2 Bass kernel for ProbSparse multi-head attention (L_Q = 1).

Math: with L_Q=1 the reference's top-k/sampling machinery is identity
(top-1 of a length-1 axis is index 0 and the scatter overwrites the whole
context), so the computation reduces to single-query attention:

  out[b] = concat_h( softmax((q Wq)_h . (k Wk)_h^T / 8) @ (v Wv)_h ) @ Wo + bo

Key algebraic restructuring (L_Q = 1 => low rank):
  scores[b,h,s] = k[b,s,:] . r[b,h,:]      with r[b,h] = Wk_h @ (qh[b,h]/8)
  w[b,h,:]      = sum_s attn[b,h,s] v[b,s,:]
  out[b]        = rowsum_h(masked((w/Z) Wv)) @ Wo + bo

so the big k/v tensors are consumed by exactly one streaming pass each and
never projected through the weight matrices (64x fewer FLOPs).

Sharding: data-parallel over batch, 2 batches per core, 8 cores.

Hardware sync-wait limits shape the implementation: transpose-mode matmuls
and a few DVE instruction structs accept only ONE semaphore wait, so all
transposes run on the PE (whose PSUM slot reuse is self-observed), constant
DMAs are absorbed into the DVE clock with touch-copies, and the DMA
completion semaphores are collapsed to one lane per DGE type.
"""

import sys

sys.path.insert(0, "/opt/trn_rl_repo")

import numpy as np
import ml_dtypes

import concourse.bass as bass
import concourse.mybir as mybir
from bass_rust import add_dep_helper
import concourse.tile_sem_assignment as _tsa
from concourse.tile import TileContext
from concourse import bass_utils


# ---- framework patch: the kernel-tail drain aggregates one semaphore wait
# per active proc, exceeding the 1-wait DRAIN encoding. Split the waits
# across a chain of single-wait drains.
from concourse.tile import TileContext as _TC
from concourse.vector_clock import ScopedClock as _SC

def _split_drain_and_barrier(self, tick_clock, wait_clock):
    drain_inst = self.nc.sync.drain()
    wait_clock.add_sem_waits(drain_inst.ins, _SC({None: tick_clock.global_clock}))
    si = drain_inst.ins.sync_info
    if si is not None and si.on_wait and len(si.on_wait) > 1:
        waits = list(si.on_wait)
        si.on_wait = waits[:1]
        for w in waits[1:]:
            d2 = self.nc.sync.drain()
            s2 = d2.ins.sync_info
            if s2 is None:
                d2.ins.sync_info = type(si)(on_wait=[w], on_update=[])
            else:
                s2.on_wait = [w]
    self.nc.all_engine_barrier()
    assert self.sems is not None
    popped = self.nc._tile_sem_poison_stack.pop()
    assert popped is self._sem_poison
    self.nc.clear_and_free_semaphores(list(self.sems.allocated().values()))
    self.nc.all_engine_barrier()

_TC._drain_and_barrier = _split_drain_and_barrier

B, H, DH, HID, LK = 16, 16, 64, 1024, 4096
NCORES = 8
BL = B // NCORES            # batches per core
NSB = LK // 512             # 8 superblocks of 512 seq positions
NCH = HID // 128            # 8 hidden chunks

f32 = mybir.dt.float32
bf16 = mybir.dt.bfloat16
FT = mybir.ActivationFunctionType
AX = mybir.AxisListType

bf16_np = ml_dtypes.bfloat16


def build_nc():
    # one DMA-completion semaphore lane per DGE type: consumers then never
    # accumulate multi-lane DMA waits (several instruction structs allow
    # only 1-2 sync waits).
    _tsa.NUM_HWDGE_SEMS = 1
    _tsa.NUM_SWDGE_GLOBAL_SEMS = 1

    nc = bass.Bass("TRN2")

    k_loc = nc.dram_tensor("k_loc", [BL, LK, HID], f32, kind="ExternalInput")
    v_loc = nc.dram_tensor("v_loc", [BL, LK, HID], f32, kind="ExternalInput")
    qT_d = nc.dram_tensor("qT", [HID, BL], bf16, kind="ExternalInput")
    Wq_d = nc.dram_tensor("Wq", [HID, HID], f32, kind="ExternalInput")
    WkT_d = nc.dram_tensor("WkT", [HID, HID], f32, kind="ExternalInput")
    Wv_d = nc.dram_tensor("Wv", [HID, HID], f32, kind="ExternalInput")
    Wo_d = nc.dram_tensor("Wo", [HID, HID], f32, kind="ExternalInput")
    bqT_d = nc.dram_tensor("bqT", [128, NCH], f32, kind="ExternalInput")
    bvT_d = nc.dram_tensor("bvT", [128, NCH], f32, kind="ExternalInput")
    boB_d = nc.dram_tensor("boB", [1, HID], bf16, kind="ExternalInput")
    one1_d = nc.dram_tensor("one1", [1, 1], bf16, kind="ExternalInput")
    idb_d = nc.dram_tensor("identb", [128, 128], bf16, kind="ExternalInput")
    mask_d = nc.dram_tensor("mask", [H, HID], bf16, kind="ExternalInput")
    out_d = nc.dram_tensor("out_loc", [BL, HID], f32, kind="ExternalOutput")

    with TileContext(nc) as tc:
        with tc.tile_pool(name="main", bufs=1) as mp, \
             tc.tile_pool(name="ps", bufs=1, space="PSUM") as pp:

            # ---- constants ----
            mask_sb = mp.tile([H, HID], bf16, tag="mask")
            nc.sync.dma_start(out=mask_sb, in_=mask_d[:, :])
            bqT = mp.tile([128, NCH], f32, tag="bqT")
            nc.sync.dma_start(out=bqT, in_=bqT_d[:, :])
            bvT = mp.tile([128, NCH], f32, tag="bvT")
            nc.sync.dma_start(out=bvT, in_=bvT_d[:, :])
            boB = mp.tile([1, HID], bf16, tag="boB")
            nc.sync.dma_start(out=boB, in_=boB_d[:, :])
            one1 = mp.tile([1, 1], bf16, tag="one1")
            nc.sync.dma_start(out=one1, in_=one1_d[:, :])
            # identity arrives on the SWDGE lane, same as k tiles: PE
            # transposes then carry exactly one DMA wait.
            idb = mp.tile([128, 128], bf16, tag="idb")
            nc.sync.dma_start(out=idb, in_=idb_d[:, :])

            # absorb constant-DMA semaphores into the DVE vector clock
            scratch = mp.tile([128, 8], f32, tag="scratch")
            scratch2 = mp.tile([1, 8], f32, tag="scratch2")

            scratch3 = mp.tile([1, 8], f32, tag="scratch3")
            nc.vector.tensor_copy(scratch[:, 0:1], bqT[:, 0:1])
            nc.vector.tensor_copy(scratch[:, 1:2], bvT[:, 1:2])
            nc.vector.tensor_copy(scratch[0:1, 2:3], boB[0:1, 0:1])
            nc.vector.tensor_copy(scratch[0:16, 3:4], mask_sb[0:16, 0:1])

            rT_sb = mp.tile([128, NCH, BL * H], bf16, tag="rT")
            Wv_sb = mp.tile([128, NCH, HID], bf16, tag="Wv")
            Wo_sb = mp.tile([128, NCH, HID], bf16, tag="Wo")

            # ---- one-time setup: qh, r (kept resident: releasing the pool
            # would create SBUF zone-reuse deps that overflow the 1-wait
            # budget of downstream instructions) ----
            if True:
                sp = mp
                qT_sb = sp.tile([128, NCH, BL], bf16, tag="qT")
                nc.sync.dma_start(out=qT_sb, in_=qT_d[:, :].rearrange("(ch p) b -> p ch b", p=128))
                Wq_sb = sp.tile([128, NCH, HID], bf16, tag="Wq")
                WkT_sb = sp.tile([128, NCH, HID], bf16, tag="WkT")
                for wdst, wsrc in ((Wq_sb, Wq_d), (WkT_sb, WkT_d)):
                    for ch in range(NCH):
                        for hf in range(2):
                            wst = mp.tile([128, 512], f32, tag="wst", bufs=4)
                            nc.scalar.dma_start(
                                out=wst,
                                in_=wsrc[ch * 128:(ch + 1) * 128,
                                         hf * 512:(hf + 1) * 512])
                            nc.scalar.copy(wdst[:, ch, hf * 512:(hf + 1) * 512], wst)
                            nc.scalar.copy(scratch3[0:1, hf:hf + 1],
                                           wdst[0:1, ch, hf * 512:hf * 512 + 1])

                # qhT[hd, b] = sum_c Wq[c, hd] qT[c, b]   (pre-scaled by 1/8)
                psum_qhT = pp.tile([128, NCH * BL], f32, tag="w", bufs=1)
                for m in range(NCH):
                    for ch in range(NCH):
                        nc.tensor.matmul(
                            psum_qhT[:, m * BL:(m + 1) * BL],
                            Wq_sb[:, ch, m * 128:(m + 1) * 128],
                            qT_sb[:, ch, :],
                            start=(ch == 0), stop=(ch == NCH - 1))
                qhT_sb = sp.tile([128, NCH, BL], f32, tag="qhT")
                for m in range(NCH):
                    nc.vector.tensor_scalar_add(
                        qhT_sb[:, m, :], psum_qhT[:, m * BL:(m + 1) * BL],
                        bqT[:, m:m + 1])

                # Qt: block-diag expansion [hd, (ch, b, h)], h == head(hd)
                Qt_sb = sp.tile([128, NCH, BL, H], bf16, tag="Qt")
                nc.vector.memset(Qt_sb, 0.0)
                for m in range(NCH):
                    for g in range(2):
                        h = 2 * m + g
                        nc.vector.tensor_copy(
                            Qt_sb[g * 64:(g + 1) * 64, m, :, h],
                            qhT_sb[g * 64:(g + 1) * 64, m, :])

                # rT[c, (b h)] = sum_hd WkT[hd, c] Qt[hd, (b h)]
                for cj in range(NCH):
                    psum_rT = pp.tile([128, BL * H], f32, tag="w", bufs=1)
                    for ch in range(NCH):
                        nc.tensor.matmul(
                            psum_rT,
                            WkT_sb[:, ch, cj * 128:(cj + 1) * 128],
                            Qt_sb[:, ch, :, :],
                            start=(ch == 0), stop=(ch == NCH - 1))
                    nc.vector.tensor_copy(rT_sb[:, cj, :], psum_rT)

            for wdst, wsrc in ((Wv_sb, Wv_d), (Wo_sb, Wo_d)):
                for ch in range(NCH):
                    for hf in range(2):
                        wst = mp.tile([128, 512], f32, tag="wst", bufs=4)
                        nc.scalar.dma_start(
                            out=wst,
                            in_=wsrc[ch * 128:(ch + 1) * 128,
                                     hf * 512:(hf + 1) * 512])
                        nc.scalar.copy(wdst[:, ch, hf * 512:(hf + 1) * 512], wst)
                        nc.scalar.copy(scratch3[0:1, hf:hf + 1],
                                       wdst[0:1, ch, hf * 512:hf * 512 + 1])




            # ---- streaming batches ----
            last_tail_mm = None
            last_vtile = [None]
            for bl in range(BL):
                # scores phase ------------------------------------------------
                scores_sb = mp.tile([H, LK], f32, tag="scores", bufs=1)
                m8 = mp.tile([H, NSB], f32, tag="m8", bufs=2)
                for jj in range(NSB):
                    if bl > 0 and jj == 0:
                        # ratchet the ACT-sequencer's observed clock up to the
                        # previous batch's last cast before issuing stream DMAs
                        nc.scalar.copy(scratch2[0:1, 0:1],
                                       last_vtile[0][0:1, 0, 0:1])
                        for x in range(4):
                            nc.scalar.copy(scratch2[0:1, x + 1:x + 2],
                                           scratch2[0:1, x:x + 1])
                    k_tile = mp.tile([128, 4, HID], bf16, tag="k", bufs=2)
                    for h3 in range(2):
                        # stream DMA issued from the ACT sequencer: the
                        # WAR against the slot's previous cast is then plain
                        # ACT program order, so the DMA's single wait slot
                        # holds only the (merged) DMA-lane completion tick.
                        if bl > 0 and jj == 0:
                            kf32 = mp.tile([128, 2, HID], f32, tag=f"kf32x{h3}", bufs=1)
                        else:
                            kf32 = mp.tile([128, 2, HID], f32, tag="kf32", bufs=2)
                        nc.scalar.dma_start(
                            out=kf32,
                            in_=k_loc[bl, jj * 512 + h3 * 256:jj * 512 + (h3 + 1) * 256, :]
                            .rearrange("(blk p) c -> p blk c", p=128))
                        nc.scalar.copy(k_tile[:, h3 * 2:(h3 + 1) * 2, :], kf32)
                        if bl == 0 and jj == 0:
                            nc.scalar.copy(scratch2[0:1, h3 * 4:h3 * 4 + 1],
                                           k_tile[0:1, h3 * 2, 0:1])
                            for x in range(3):
                                nc.scalar.copy(
                                    scratch2[0:1, h3 * 4 + x + 1:h3 * 4 + x + 2],
                                    scratch2[0:1, h3 * 4 + x:h3 * 4 + x + 1])
                    # absorber pair: 1-col LDWEIGHTS takes the k-DMA wait,
                    # the tiny matmul soaks up any residual cross-engine deps
                    # so the single-wait transposes only carry their psum WAW.
                    psum_s = pp.tile([33, 512], f32, tag="s", bufs=2)
                    ldw_abs = nc.tensor.ldweights(k_tile[:, 0, 0:1])
                    # absorber matmul doubles as the psum slot's first writer,
                    # carrying its WAR on the multi-wait-capable MM pair
                    dmy = nc.tensor.matmul(psum_s[32:33, 0:1], k_tile[:, 0, 0:1],
                                           k_tile[:, 0, 0:1], start=True, stop=True)
                    add_dep_helper(dmy.ins, ldw_abs.ins, reason="absorb-chain")
                    if jj == 0 and last_tail_mm is not None:
                        # order batch bl's first transposes after batch bl-1's
                        # final matmul so the PE clock has observed the tail
                        # copies (k data for this batch arrives later anyway)
                        add_dep_helper(dmy.ins, last_tail_mm.ins,
                                       reason="batch-boundary order")
                    kT_sb = mp.tile([128, NCH, 512], bf16, tag="kT", bufs=2)
                    for t in range(2):
                        psum_kt = pp.tile([128, 4, 4, 128], bf16, tag="tp", bufs=2)
                        for u in range(4):
                            cj = 4 * t + u
                            for blk in range(4):
                                tp_i = nc.tensor.transpose(
                                    psum_kt[:, u, blk, :],
                                    k_tile[:, blk, cj * 128:(cj + 1) * 128],
                                    idb)
                                add_dep_helper(tp_i.ins, dmy.ins, reason="absorb")
                                if jj == 0 and last_tail_mm is not None:
                                    add_dep_helper(tp_i.ins, last_tail_mm.ins,
                                                   reason="batch-boundary order")
                        nc.vector.tensor_copy(
                            kT_sb[:, 4 * t:4 * t + 4, :],
                            psum_kt.rearrange("p a b c -> p (a b c)"))
                    for cj in range(NCH):
                        nc.tensor.matmul(
                            psum_s[0:H, :],
                            rT_sb[:, cj, bl * H:(bl + 1) * H],
                            kT_sb[:, cj, :],
                            start=(cj == 0), stop=(cj == NCH - 1))
                    nc.vector.reduce_max(m8[:, jj:jj + 1], psum_s[0:H, :], axis=AX.X)
                    nc.vector.tensor_copy(scores_sb[:, jj * 512:(jj + 1) * 512], psum_s[0:H, :])
                    # chained pads: ratchet the ACT clock past the scorecopy
                    # and pad the pipeline for the next superblock's triggers
                    nc.scalar.copy(scratch3[0:1, 0:1],
                                   scores_sb[0:1, jj * 512:jj * 512 + 1])
                    for x in range(3):
                        nc.scalar.copy(scratch3[0:1, x + 1:x + 2],
                                       scratch3[0:1, x:x + 1])

                # softmax -----------------------------------------------------
                negmax = mp.tile([H, 1], f32, tag="negmax", bufs=2)
                nc.vector.reduce_max(negmax, m8, axis=AX.X, negate=True)
                attn_sb = mp.tile([H, LK], bf16, tag="attn", bufs=1)
                Zs = mp.tile([H, NSB], f32, tag="Zs", bufs=2)
                prev_pad = None
                for pad in range(4):
                    p_i = nc.scalar.copy(scratch2[0:1, pad:pad + 1],
                                         scores_sb[0:1, 0:1])
                    if prev_pad is not None:
                        add_dep_helper(p_i.ins, prev_pad.ins, reason="act-pad-chain")
                    prev_pad = p_i
                for jj in range(NSB):
                    e_i = nc.scalar.activation(
                        attn_sb[:, jj * 512:(jj + 1) * 512],
                        scores_sb[:, jj * 512:(jj + 1) * 512],
                        FT.Exp, bias=negmax, scale=1.0, accum_out=Zs[:, jj:jj + 1])
                Z = mp.tile([H, 1], f32, tag="Z", bufs=2)
                nc.vector.reduce_sum(Z, Zs, axis=AX.X)
                Zi = mp.tile([H, 1], f32, tag="Zi", bufs=2)
                zi_i = nc.vector.reciprocal(Zi, Z)
                # attn transposed on PE, 16 seq-chunks per psum tile
                ldw_at = nc.tensor.ldweights(attn_sb[0:H, 0:1])
                psum_sa = pp.tile([33, 512], f32, tag="s", bufs=2)
                dmy_at = nc.tensor.matmul(psum_sa[32:33, 0:1], attn_sb[0:H, 0:1],
                                          attn_sb[0:H, 0:1], start=True, stop=True)
                add_dep_helper(dmy_at.ins, ldw_at.ins, reason="absorb-chain")
                attnT_sb = mp.tile([128, LK // 128, H], bf16, tag="attnT", bufs=1)
                for g in range(2):
                    psum_at = pp.tile([128, 16, H], bf16, tag="tp", bufs=2)
                    for t in range(16):
                        j = g * 16 + t
                        tp_i = nc.tensor.transpose(
                            psum_at[:, t, :],
                            attn_sb[0:H, j * 128:(j + 1) * 128], idb[0:H, 0:H])
                        add_dep_helper(tp_i.ins, dmy_at.ins, reason="absorb")
                    nc.vector.tensor_copy(
                        attnT_sb[:, g * 16:(g + 1) * 16, :],
                        psum_at)

                # weighted-V phase -------------------------------------------
                psum_w = pp.tile([H, HID], f32, tag="w", bufs=1)
                for jj in range(NSB):
                    v_tile = mp.tile([128, 4, HID], bf16, tag="v", bufs=2)
                    for h3 in range(2):
                        vf32 = mp.tile([128, 2, HID], f32, tag="vf32", bufs=2)
                        nc.scalar.dma_start(
                            out=vf32,
                            in_=v_loc[bl, jj * 512 + h3 * 256:jj * 512 + (h3 + 1) * 256, :]
                            .rearrange("(blk p) c -> p blk c", p=128))
                        nc.scalar.copy(v_tile[:, h3 * 2:(h3 + 1) * 2, :], vf32)
                        last_vtile[0] = v_tile
                    psum_sv = pp.tile([33, 512], f32, tag="s", bufs=2)
                    ldw_v = nc.tensor.ldweights(v_tile[:, 0, 0:1])
                    dmy_v = nc.tensor.matmul(psum_sv[32:33, 0:1], v_tile[:, 0, 0:1],
                                             v_tile[:, 0, 0:1], start=True, stop=True)
                    add_dep_helper(dmy_v.ins, ldw_v.ins, reason="absorb-chain")
                    nc.scalar.copy(scratch3[0:1, jj % 4:jj % 4 + 1],
                                   psum_sv[32:33, 0:1])
                    for x in range(4):
                        nc.scalar.copy(scratch3[0:1, 4 + x:5 + x],
                                       scratch3[0:1, jj % 4:jj % 4 + 1])
                    for blk in range(4):
                        for hf in range(2):
                            mm_i = nc.tensor.matmul(
                                psum_w[:, hf * 512:(hf + 1) * 512],
                                attnT_sb[:, jj * 4 + blk, :],
                                v_tile[:, blk, hf * 512:(hf + 1) * 512],
                                start=(jj == 0 and blk == 0),
                                stop=(jj == NSB - 1 and blk == 3))
                            add_dep_helper(mm_i.ins, dmy_v.ins, reason="absorb")

                # tail: w -> u -> out ----------------------------------------
                w_sb = mp.tile([H, HID], bf16, tag="wsb", bufs=1)
                nc.vector.tensor_copy(w_sb, psum_w)
                ldw_w = nc.tensor.ldweights(w_sb[0:H, 0:1])
                psum_sw = pp.tile([33, 512], f32, tag="s", bufs=2)
                dmy_w = nc.tensor.matmul(psum_sw[32:33, 0:1], w_sb[0:H, 0:1],
                                         w_sb[0:H, 0:1], start=True, stop=True)
                add_dep_helper(dmy_w.ins, ldw_w.ins, reason="absorb-chain")
                psum_wt = pp.tile([128, NCH, H], bf16, tag="tp", bufs=2)
                for cj in range(NCH):
                    tp_i = nc.tensor.transpose(
                        psum_wt[:, cj, :],
                        w_sb[0:H, cj * 128:(cj + 1) * 128], idb[0:H, 0:H])
                    add_dep_helper(tp_i.ins, dmy_w.ins, reason="absorb")
                wT_sb = mp.tile([128, NCH, H], bf16, tag="wT", bufs=1)
                nc.vector.tensor_copy(wT_sb, psum_wt)

                nc.tensor.ldweights(Wv_sb[:, 0, 0:1])
                psum_u = pp.tile([H, HID], f32, tag="w")
                for cj in range(NCH):
                    for hf in range(2):
                        nc.tensor.matmul(
                            psum_u[:, hf * 512:(hf + 1) * 512],
                            wT_sb[:, cj, :],
                            Wv_sb[:, cj, hf * 512:(hf + 1) * 512],
                            start=(cj == 0), stop=(cj == NCH - 1))
                # DVE pad chain so Zi's in-engine drain settles before um
                prev = zi_i
                for pad in range(8):
                    p_i = nc.vector.tensor_copy(scratch[:, pad:pad + 1], scratch[:, 0:1])
                    add_dep_helper(p_i.ins, prev.ins, reason="dve-pad-chain")
                    prev = p_i
                um_sb = mp.tile([H, HID], bf16, tag="um", bufs=1)
                um_i = nc.vector.scalar_tensor_tensor(
                    um_sb, psum_u, Zi, mask_sb,
                    op0=mybir.AluOpType.mult, op1=mybir.AluOpType.mult)

                # transpose u_masked, then fold heads with a free-dim reduce
                ldw_um = nc.tensor.ldweights(um_sb[0:H, 0:1])
                psum_su = pp.tile([33, 512], f32, tag="s", bufs=2)
                dmy_um = nc.tensor.matmul(psum_su[32:33, 0:1], um_sb[0:H, 0:1],
                                          um_sb[0:H, 0:1], start=True, stop=True)
                add_dep_helper(dmy_um.ins, ldw_um.ins, reason="absorb-chain")
                psum_umt = pp.tile([128, NCH, H], bf16, tag="tp", bufs=2)
                for m in range(NCH):
                    tp_i = nc.tensor.transpose(
                        psum_umt[:, m, :],
                        um_sb[0:H, m * 128:(m + 1) * 128], idb[0:H, 0:H])
                    add_dep_helper(tp_i.ins, dmy_um.ins, reason="absorb")
                umT_sb = mp.tile([128, NCH, H], bf16, tag="umT", bufs=1)
                nc.vector.tensor_copy(umT_sb, psum_umt)
                uT_f = mp.tile([128, NCH], f32, tag="uTf", bufs=1)
                nc.vector.reduce_sum(uT_f, umT_sb, axis=AX.X)
                uT_sb = mp.tile([128, NCH], bf16, tag="uT", bufs=1)
                nc.vector.tensor_add(uT_sb, uT_f, bvT)

                nc.tensor.ldweights(Wo_sb[:, 0, 0:1])
                psum_o = pp.tile([1, HID], f32, tag="w")
                for m in range(NCH):
                    for hf in range(2):
                        nc.tensor.matmul(
                            psum_o[:, hf * 512:(hf + 1) * 512],
                            uT_sb[:, m:m + 1],
                            Wo_sb[:, m, hf * 512:(hf + 1) * 512],
                            start=(m == 0), stop=False)
                for hf in range(2):
                    mm_i = nc.tensor.matmul(
                        psum_o[:, hf * 512:(hf + 1) * 512],
                        one1, boB[0:1, hf * 512:(hf + 1) * 512],
                        start=False, stop=True)
                # output tail on ACT: copy + DMA-issue share the ACT queue so
                # the DMA's only wait is its lane tick
                out_sb = mp.tile([1, HID], f32, tag="osb", bufs=1)
                nc.scalar.copy(out_sb, psum_o)
                nc.scalar.copy(scratch2[0:1, 0:1], out_sb[0:1, 0:1])
                for x in range(3):
                    nc.scalar.copy(scratch2[0:1, x + 1:x + 2],
                                   scratch2[0:1, x:x + 1])
                nc.scalar.dma_start(out=out_d[bl:bl + 1, :], in_=out_sb)
                last_tail_mm = mm_i

    return nc


def make_in_maps(q, k, v, Wq, bq, Wv, bv, Wo, bo, Wk):
    scale = DH ** -0.5
    mask = np.zeros((H, HID), dtype=np.float32)
    for h in range(H):
        mask[h, h * DH:(h + 1) * DH] = 1.0
    shared = {
        "Wq": np.ascontiguousarray(Wq),
        "WkT": np.ascontiguousarray(Wk.T),
        "Wv": np.ascontiguousarray(Wv), "Wo": np.ascontiguousarray(Wo),
        "bqT": np.ascontiguousarray((bq * scale).reshape(NCH, 128).T),
        "bvT": np.ascontiguousarray(bv.reshape(NCH, 128).T),
        "boB": np.ascontiguousarray(bo.reshape(1, HID)).astype(bf16_np),
        "one1": np.ones((1, 1), np.float32).astype(bf16_np),
        "identb": np.eye(128, dtype=np.float32).astype(bf16_np),
        "mask": mask.astype(bf16_np),
    }
    in_maps = []
    for c in range(NCORES):
        sl = slice(BL * c, BL * (c + 1))
        in_maps.append({
            "k_loc": np.ascontiguousarray(k[sl]),
            "v_loc": np.ascontiguousarray(v[sl]),
            "qT": np.ascontiguousarray((q[sl] * scale).T).astype(bf16_np),
            **shared,
        })
    return in_maps


_nc_cache = None


def kernel(q, k, v, index_sample, Wq, bq, Wk, bk, Wv, bv, Wo, bo):
    global _nc_cache
    q, k, v = np.asarray(q, np.float32), np.asarray(k, np.float32), np.asarray(v, np.float32)
    Wq, bq = np.asarray(Wq, np.float32), np.asarray(bq, np.float32)
    Wk = np.asarray(Wk, np.float32)
    Wv, bv = np.asarray(Wv, np.float32), np.asarray(bv, np.float32)
    Wo, bo = np.asarray(Wo, np.float32), np.asarray(bo, np.float32)
    # bk provably cancels in the softmax (constant shift per (b, h) row).

    if _nc_cache is None:
        _nc_cache = build_nc()
    nc = _nc_cache
    in_maps = make_in_maps(q, k, v, Wq, bq, Wv, bv, Wo, bo, Wk)
    res = bass_utils.run_bass_kernel_spmd(nc, in_maps, core_ids=list(range(NCORES)))
    out = np.concatenate([r["out_loc"] for r in res.results], axis=0)
    return np.ascontiguousarray(out.astype(np.float32))



# revision 15
# speedup vs baseline: 2.8646x; 2.8646x over previous
"""Trainium2 Bass kernel for ProbSparse multi-head attention (L_Q = 1).

Math: with L_Q=1 the reference's top-k/sampling machinery is identity
(top-1 of a length-1 axis is index 0 and the scatter overwrites the whole
context), so the computation reduces to single-query attention:

  out[b] = concat_h( softmax((q Wq)_h . (k Wk)_h^T / 8) @ (v Wv)_h ) @ Wo + bo

Key algebraic restructuring (L_Q = 1 => low rank):
  scores[b,h,s] = k[b,s,:] . r[b,h,:]      with r[b,h] = Wk_h @ (qh[b,h]/8)
  w[b,h,:]      = sum_s attn[b,h,s] v[b,s,:]
  out[b]        = rowsum_h(masked((w/Z) Wv)) @ Wo + bo

so the big k/v tensors are consumed by exactly one streaming pass each and
never projected through the weight matrices (64x fewer FLOPs).

v2: k arrives pre-transposed (kT, hidden-major) and pre-cast to bf16 from
the host, v pre-cast to bf16, weights pre-cast to bf16.  This removes all
device-side f32->bf16 casts and all PE transposes of k (the v1 bottleneck:
~500us of PE occupancy and 84MB of DMA drop to ~190us PE / 38MB DMA).
The tail (w -> u -> out) is batched over both local batches so Wv / Wo
stream through the PE once per core instead of once per batch.

Sharding: data-parallel over batch, 2 batches per core, 8 cores.

Sync-wait limits (some instruction encodings accept a single semaphore
wait) are handled as in v1: PE transposes are preceded by an
absorber ldweights+matmul pair that soaks up cross-engine waits, and DVE
touch-copies of DMA'd constants early in the program teach the DVE clock
about those DMA ticks before scalar_tensor_tensor needs them.
"""

import sys

sys.path.insert(0, "/opt/trn_rl_repo")

import numpy as np
import ml_dtypes

import concourse.bass as bass
import concourse.mybir as mybir
from bass_rust import add_dep_helper
import concourse.tile_sem_assignment as _tsa
from concourse.tile import TileContext
from concourse import bass_utils


# ---- framework patch: the kernel-tail drain aggregates one semaphore wait
# per active proc, exceeding the 1-wait DRAIN encoding. Split the waits
# across a chain of single-wait drains.
from concourse.tile import TileContext as _TC
from concourse.vector_clock import ScopedClock as _SC

def _split_drain_and_barrier(self, tick_clock, wait_clock):
    drain_inst = self.nc.sync.drain()
    wait_clock.add_sem_waits(drain_inst.ins, _SC({None: tick_clock.global_clock}))
    si = drain_inst.ins.sync_info
    if si is not None and si.on_wait and len(si.on_wait) > 1:
        waits = list(si.on_wait)
        si.on_wait = waits[:1]
        for w in waits[1:]:
            d2 = self.nc.sync.drain()
            s2 = d2.ins.sync_info
            if s2 is None:
                d2.ins.sync_info = type(si)(on_wait=[w], on_update=[])
            else:
                s2.on_wait = [w]
    self.nc.all_engine_barrier()
    assert self.sems is not None
    popped = self.nc._tile_sem_poison_stack.pop()
    assert popped is self._sem_poison
    self.nc.clear_and_free_semaphores(list(self.sems.allocated().values()))
    self.nc.all_engine_barrier()

_TC._drain_and_barrier = _split_drain_and_barrier

B, H, DH, HID, LK = 16, 16, 64, 1024, 4096
NCORES = 8
BL = B // NCORES            # batches per core
NCH = HID // 128            # 8 hidden chunks
NT = LK // 1024             # 4 stream tiles of 1024 seq positions
H2 = BL * H                 # packed (b,h) rows for the r projection
HS = 64                     # tail stacking: batch b at partitions b*32

f32 = mybir.dt.float32
bf16 = mybir.dt.bfloat16
FT = mybir.ActivationFunctionType
AX = mybir.AxisListType

bf16_np = ml_dtypes.bfloat16


def build_nc():
    # one DMA-completion semaphore lane per DGE type: consumers then never
    # accumulate multi-lane DMA waits (several instruction structs allow
    # only 1-2 sync waits).
    _tsa.NUM_HWDGE_SEMS = 1
    _tsa.NUM_SWDGE_GLOBAL_SEMS = 1

    nc = bass.Bass("TRN2")

    kT_d = nc.dram_tensor("kT_loc", [BL, HID, LK], bf16, kind="ExternalInput")
    v_d = nc.dram_tensor("v_loc", [BL, LK, HID], bf16, kind="ExternalInput")
    qT_d = nc.dram_tensor("qT", [HID, BL], bf16, kind="ExternalInput")
    Wq_d = nc.dram_tensor("Wq", [HID, HID], bf16, kind="ExternalInput")
    WkT_d = nc.dram_tensor("WkT", [HID, HID], bf16, kind="ExternalInput")
    Wv_d = nc.dram_tensor("Wv", [HID, HID], bf16, kind="ExternalInput")
    Wo_d = nc.dram_tensor("Wo", [HID, HID], bf16, kind="ExternalInput")
    bqB_d = nc.dram_tensor("bqB", [1, HID], bf16, kind="ExternalInput")
    bvT_d = nc.dram_tensor("bvT", [128, NCH], f32, kind="ExternalInput")
    boB_d = nc.dram_tensor("boB", [1, HID], bf16, kind="ExternalInput")
    one12_d = nc.dram_tensor("one12", [1, BL], bf16, kind="ExternalInput")
    one14_d = nc.dram_tensor("one14", [1, 4], bf16, kind="ExternalInput")
    idb_d = nc.dram_tensor("identb", [128, 128], bf16, kind="ExternalInput")
    mask2_d = nc.dram_tensor("mask2", [HS, HID], bf16, kind="ExternalInput")
    out_d = nc.dram_tensor("out_loc", [BL, HID], f32, kind="ExternalOutput")

    with TileContext(nc) as tc:
        with tc.tile_pool(name="main", bufs=1) as mp, \
             tc.tile_pool(name="ps", bufs=1, space="PSUM") as pp:

            # ---- constants (scalar HWDGE queue) ----
            idb = mp.tile([128, 128], bf16, tag="idb")
            nc.scalar.dma_start(out=idb, in_=idb_d[:, :])
            mask2 = mp.tile([HS, HID], bf16, tag="mask2")
            nc.scalar.dma_start(out=mask2, in_=mask2_d[:, :])
            boB = mp.tile([1, HID], bf16, tag="boB")
            nc.scalar.dma_start(out=boB, in_=boB_d[:, :])
            one14 = mp.tile([1, 4], bf16, tag="one14")
            nc.scalar.dma_start(out=one14, in_=one14_d[:, :])
            bvT = mp.tile([128, NCH], f32, tag="bvT")
            nc.scalar.dma_start(out=bvT, in_=bvT_d[:, :])

            # DVE touch-copies: teach the DVE clock the const-DMA ticks so
            # later 1-wait DVE structs (scalar_tensor_tensor) don't need a
            # DMA wait slot of their own.
            scratch = mp.tile([128, 8], f32, tag="scratch")
            nc.vector.tensor_copy(scratch[0:HS, 0:1], mask2[:, 0:1])
            nc.vector.tensor_copy(scratch[:, 1:2], bvT[:, 0:1])
            scratch2 = mp.tile([1, 8], f32, tag="scratch2")
            rT_sb = mp.tile([128, NCH, H2], bf16, tag="rT")

            # dedicated never-read psum tile: every absorber dummy writes
            # here, so each dummy carries only the PE WAW-drain wait of its
            # predecessor (a chain) and no DVE WAR.
            dmy_ps = pp.tile([1, 64], f32, tag="dmy")

            # ---- setup: qh = Wq^T qT (+bq), r = WkT^T Qt ----
            # setup-only SBUF lives in a nested pool so its 37KB/partition is
            # reused by the stream tiles afterwards.
            sp_ctx = tc.tile_pool(name="setup", bufs=1)
            sp = sp_ctx.__enter__()
            qT_sb = sp.tile([128, NCH, BL], bf16, tag="qT")
            nc.scalar.dma_start(
                out=qT_sb, in_=qT_d[:, :].rearrange("(ch p) b -> p ch b", p=128))
            Wq_sb = sp.tile([128, NCH, HID], bf16, tag="Wq")
            nc.scalar.dma_start(
                out=Wq_sb, in_=Wq_d[:, :].rearrange("(ch p) h -> p ch h", p=128))
            WkT_sb = sp.tile([128, NCH, HID], bf16, tag="WkT")
            nc.scalar.dma_start(
                out=WkT_sb, in_=WkT_d[:, :].rearrange("(ch p) h -> p ch h", p=128))
            bqB = sp.tile([1, HID], bf16, tag="bqB")
            nc.scalar.dma_start(out=bqB, in_=bqB_d[:, :])
            one12 = sp.tile([1, BL], bf16, tag="one12")
            nc.scalar.dma_start(out=one12, in_=one12_d[:, :])
            # qh[b, :]: lhsT = qT chunk [128, BL], moving = Wq chunk.
            psum_qh = pp.tile([32, HID], f32, tag="w")
            for ch in range(NCH):
                for hf in range(2):
                    nc.tensor.matmul(
                        psum_qh[0:BL, hf * 512:(hf + 1) * 512],
                        qT_sb[:, ch, :],
                        Wq_sb[:, ch, hf * 512:(hf + 1) * 512],
                        start=(ch == 0), stop=False)
            for hf in range(2):
                nc.tensor.matmul(
                    psum_qh[0:BL, hf * 512:(hf + 1) * 512],
                    one12, bqB[0:1, hf * 512:(hf + 1) * 512],
                    start=False, stop=(hf == 1))
            qh_sb = sp.tile([BL, HID], bf16, tag="qh")
            nc.vector.tensor_copy(qh_sb, psum_qh[0:BL, :])

            # transpose qh -> qhT [hd, b] (PE, absorber pattern)
            ldw_q = nc.tensor.ldweights(qh_sb[0:BL, 0:1])
            dmy_q = nc.tensor.matmul(dmy_ps[0:1, 0:BL], qh_sb[0:BL, 0:1],
                                     idb[0:BL, 0:BL], start=True, stop=True)
            add_dep_helper(dmy_q.ins, ldw_q.ins, reason="absorb-chain")
            psum_qt = pp.tile([128, NCH, BL], bf16, tag="tp")
            for ch in range(NCH):
                tp_i = nc.tensor.transpose(
                    psum_qt[:, ch, :],
                    qh_sb[0:BL, ch * 128:(ch + 1) * 128], idb[0:BL, 0:BL])
                add_dep_helper(tp_i.ins, dmy_q.ins, reason="absorb")
            qhT_sb = sp.tile([128, NCH, BL], bf16, tag="qhT")
            nc.vector.tensor_copy(qhT_sb, psum_qt)

            # Qt: block-diag expansion [hd, (ch, b, h)], h == head(hd)
            Qt_sb = sp.tile([128, NCH, BL, H], bf16, tag="Qt")
            nc.vector.memset(Qt_sb, 0.0)
            for m in range(NCH):
                for g in range(2):
                    h = 2 * m + g
                    nc.vector.tensor_copy(
                        Qt_sb[g * 64:(g + 1) * 64, m, :, h],
                        qhT_sb[g * 64:(g + 1) * 64, m, :])

            # rTT[(b h), c] = sum_hd Qt[hd, (b h)] WkT[hd, c]
            psum_rTT = pp.tile([32, HID], f32, tag="w")
            ldw_wk = nc.tensor.ldweights(WkT_sb[:, 0, 0:1])
            first_rtt = [True]
            for ch in range(NCH):
                for hf in range(2):
                    mm = nc.tensor.matmul(
                        psum_rTT[0:H2, hf * 512:(hf + 1) * 512],
                        Qt_sb[:, ch, :, :],
                        WkT_sb[:, ch, hf * 512:(hf + 1) * 512],
                        start=(ch == 0), stop=(ch == NCH - 1))
                    if first_rtt[0]:
                        add_dep_helper(mm.ins, ldw_wk.ins, reason="absorb")
                        first_rtt[0] = False
            rTT_sb = sp.tile([H2, HID], bf16, tag="rTT")
            nc.vector.tensor_copy(rTT_sb, psum_rTT[0:H2, :])

            # transpose rTT -> rT [c, (b h)]
            ldw_r = nc.tensor.ldweights(rTT_sb[0:H2, 0:1])
            dmy_r = nc.tensor.matmul(dmy_ps[0:1, 0:H2], rTT_sb[0:H2, 0:1],
                                     idb[0:H2, 0:H2], start=True, stop=True)
            add_dep_helper(dmy_r.ins, ldw_r.ins, reason="absorb-chain")
            psum_rt = pp.tile([128, NCH, H2], bf16, tag="tp")
            for cj in range(NCH):
                tp_i = nc.tensor.transpose(
                    psum_rt[:, cj, :],
                    rTT_sb[0:H2, cj * 128:(cj + 1) * 128], idb[0:H2, 0:H2])
                add_dep_helper(tp_i.ins, dmy_r.ins, reason="absorb")
            nc.vector.tensor_copy(rT_sb, psum_rt)
            sp_ctx.__exit__(None, None, None)
            # PE marker into the dmy_ps corner, then an ACT touch of that
            # corner: the touch carries one PE RAW wait and ratchets the ACT
            # clock over all setup PE work.
            nc.tensor.matmul(dmy_ps[0:1, 48:49], rTT_sb[0:1, 0:1],
                             rTT_sb[0:1, 0:1], start=True, stop=True)
            nc.scalar.copy(scratch2[0:1, 0:1], dmy_ps[0:1, 48:49])

            # ---- streaming batches ----
            psum_w = [None, None]
            Zi2 = mp.tile([HS, 1], f32, tag="Zi2")
            nc.vector.memset(Zi2, 0.0)
            last_psum_at = [None]
            attnT_sb_prev = [None]
            for bl in range(BL):
                if bl > 0:
                    # ratchet ACT past batch bl-1's V phase (the marker) so
                    # this batch's kt/vt triggers carry only their DMA WAW.
                    nc.scalar.copy(scratch2[0:1, 1:2], dmy_ps[0:1, 48:49])
                # scores phase ------------------------------------------------
                scores_sb = mp.tile([H, LK], bf16, tag="scores", bufs=1)
                m8 = mp.tile([H, 2 * NT], f32, tag="m8", bufs=2)
                for t in range(NT):
                    kt = mp.tile([128, NCH, 1024], bf16, tag="kt", bufs=4)
                    nc.scalar.dma_start(
                        out=kt,
                        in_=kT_d[bl, :, t * 1024:(t + 1) * 1024]
                        .rearrange("(ch p) s -> p ch s", p=128))
                    ldw_k = nc.tensor.ldweights(kt[:, 0, 0:1])
                    prev_mm = None
                    for sh in range(2):
                        blk = t * 2 + sh
                        psum_s = pp.tile([33, 512], f32, tag="s")
                        dmy = nc.tensor.matmul(
                            dmy_ps[0:1, 0:1], kt[:, 0, 0:1], kt[:, 0, 0:1],
                            start=True, stop=True)
                        add_dep_helper(dmy.ins, (ldw_k if sh == 0 else prev_mm).ins,
                                       reason="absorb-chain")
                        for cj in range(NCH):
                            mm = nc.tensor.matmul(
                                psum_s[0:H, :],
                                rT_sb[:, cj, bl * H:(bl + 1) * H],
                                kt[:, cj, sh * 512:(sh + 1) * 512],
                                start=(cj == 0), stop=(cj == NCH - 1))
                            if cj == 0:
                                add_dep_helper(mm.ins, dmy.ins, reason="absorb")
                            prev_mm = mm
                        nc.vector.reduce_max(m8[:, blk:blk + 1], psum_s[0:H, :],
                                             axis=AX.X)
                        nc.vector.tensor_copy(
                            scores_sb[:, blk * 512:(blk + 1) * 512], psum_s[0:H, :])

                # scores-complete marker + ACT ratchet: placed a whole phase
                # upstream of the next batch's kt triggers so the scheduler's
                # DMA hoisting cannot lift them above it.
                nc.tensor.matmul(dmy_ps[0:1, 48:49], kt[0:1, 0, 0:1],
                                 kt[0:1, 0, 0:1], start=True, stop=True)
                nc.scalar.copy(scratch2[0:1, 4 + bl:5 + bl],
                               dmy_ps[0:1, 48:49])

                # softmax -----------------------------------------------------
                negmax = mp.tile([H, 1], f32, tag="negmax", bufs=2)
                nc.vector.reduce_max(negmax, m8, axis=AX.X, negate=True)
                attn_sb = mp.tile([H, LK], bf16, tag="attn", bufs=2)
                Zs = mp.tile([H, 2 * NT], f32, tag="Zs", bufs=2)
                for jj in range(2 * NT):
                    nc.scalar.activation(
                        attn_sb[:, jj * 512:(jj + 1) * 512],
                        scores_sb[:, jj * 512:(jj + 1) * 512],
                        FT.Exp, bias=negmax, scale=1.0,
                        accum_out=Zs[:, jj:jj + 1])
                Z = mp.tile([H, 1], f32, tag="Z", bufs=2)
                nc.vector.reduce_sum(Z, Zs, axis=AX.X)
                nc.vector.reciprocal(Zi2[bl * 32:bl * 32 + H, :], Z)

                # attn transposed on PE -> attnT [s, h]
                ldw_at = nc.tensor.ldweights(attn_sb[0:H, 0:1])
                dmy_at = nc.tensor.matmul(dmy_ps[0:1, 0:H], attn_sb[0:H, 0:1],
                                          idb[0:H, 0:H], start=True, stop=True)
                add_dep_helper(dmy_at.ins, ldw_at.ins, reason="absorb-chain")
                attnT_sb = mp.tile([128, LK // 128, H], bf16, tag="attnT", bufs=1)
                for g in range(2):
                    psum_at = pp.tile([128, 16, H], bf16, tag="tp")
                    for tt in range(16):
                        j = g * 16 + tt
                        tp_i = nc.tensor.transpose(
                            psum_at[:, tt, :],
                            attn_sb[0:H, j * 128:(j + 1) * 128], idb[0:H, 0:H])
                        add_dep_helper(tp_i.ins, dmy_at.ins, reason="absorb")
                    nc.vector.tensor_copy(
                        attnT_sb[:, g * 16:(g + 1) * 16, :], psum_at)
                    last_psum_at[0] = psum_at
                attnT_sb_prev[0] = attnT_sb

                # weighted-V phase -------------------------------------------
                pw = pp.tile([32, HID], f32, tag="w")
                psum_w[bl] = pw
                for t in range(NT):
                    vt = mp.tile([128, NCH, HID], bf16, tag="vt", bufs=4)
                    nc.scalar.dma_start(
                        out=vt,
                        in_=v_d[bl, t * 1024:(t + 1) * 1024, :]
                        .rearrange("(blk p) c -> p blk c", p=128))
                    ldw_v = nc.tensor.ldweights(vt[:, 0, 0:1])
                    dmy_v = nc.tensor.matmul(
                        dmy_ps[0:1, 0:1], vt[:, 0, 0:1], vt[:, 0, 0:1],
                        start=True, stop=True)
                    add_dep_helper(dmy_v.ins, ldw_v.ins, reason="absorb-chain")
                    for blk in range(NCH):
                        for hf in range(2):
                            mm = nc.tensor.matmul(
                                pw[0:H, hf * 512:(hf + 1) * 512],
                                attnT_sb[:, t * NCH + blk, :],
                                vt[:, blk, hf * 512:(hf + 1) * 512],
                                start=(t == 0 and blk == 0),
                                stop=(t == NT - 1 and blk == NCH - 1))
                            if blk == 0 and hf == 0:
                                add_dep_helper(mm.ins, dmy_v.ins, reason="absorb")
                # V-phase-complete marker for the next batch's ACT ratchet
                nc.tensor.matmul(dmy_ps[0:1, 48:49], vt[0:1, 0, 0:1],
                                 vt[0:1, 0, 0:1], start=True, stop=True)

            # ---- batched tail: w -> u -> out for both batches at once ----
            w2_sb = mp.tile([HS, HID], bf16, tag="w2")
            nc.vector.memset(w2_sb, 0.0)
            for bl in range(BL):
                nc.vector.tensor_copy(w2_sb[bl * 32:bl * 32 + H, :],
                                      psum_w[bl][0:H, :])
            # ratchet ACT past batch 1's V accumulation, then pull the tail
            # weights into recycled stream-tile slots.
            nc.scalar.copy(scratch2[0:1, 3:4], dmy_ps[0:1, 48:49])
            Wv_sb = mp.tile([128, NCH, HID], bf16, tag="kt", bufs=4)
            nc.scalar.dma_start(
                out=Wv_sb, in_=Wv_d[:, :].rearrange("(ch p) h -> p ch h", p=128))
            Wo_sb = mp.tile([128, NCH, HID], bf16, tag="vt", bufs=4)
            nc.scalar.dma_start(
                out=Wo_sb, in_=Wo_d[:, :].rearrange("(ch p) h -> p ch h", p=128))
            ldw_w = nc.tensor.ldweights(w2_sb[0:HS, 0:1])
            dmy_w = nc.tensor.matmul(dmy_ps[0:1, 0:HS], w2_sb[0:HS, 0:1],
                                     idb[0:HS, 0:HS], start=True, stop=True)
            add_dep_helper(dmy_w.ins, ldw_w.ins, reason="absorb-chain")
            psum_wt = pp.tile([128, NCH, HS], bf16, tag="tp")
            for cj in range(NCH):
                tp_i = nc.tensor.transpose(
                    psum_wt[:, cj, :],
                    w2_sb[0:HS, cj * 128:(cj + 1) * 128], idb[0:HS, 0:HS])
                add_dep_helper(tp_i.ins, dmy_w.ins, reason="absorb")
            wT_sb = mp.tile([128, NCH, HS], bf16, tag="wT")
            nc.vector.tensor_copy(wT_sb, psum_wt)

            psum_u = pp.tile([HS, HID], f32, tag="w")
            ldw_wv = nc.tensor.ldweights(Wv_sb[:, 0, 0:1])
            dmy_u = nc.tensor.matmul(dmy_ps[0:1, 0:1], wT_sb[:, 0, 0:1],
                                     wT_sb[:, 0, 0:1], start=True, stop=True)
            add_dep_helper(dmy_u.ins, ldw_wv.ins, reason="absorb-chain")
            first_u = [True]
            for cj in range(NCH):
                for hf in range(2):
                    mm = nc.tensor.matmul(
                        psum_u[0:HS, hf * 512:(hf + 1) * 512],
                        wT_sb[:, cj, :],
                        Wv_sb[:, cj, hf * 512:(hf + 1) * 512],
                        start=(cj == 0), stop=(cj == NCH - 1))
                    if first_u[0]:
                        add_dep_helper(mm.ins, dmy_u.ins, reason="absorb")
                        first_u[0] = False
            um2_sb = mp.tile([HS, HID], bf16, tag="um2")
            nc.vector.tensor_copy(scratch[0:1, 3:4], psum_u[0:1, 0:1])
            nc.vector.scalar_tensor_tensor(
                um2_sb, psum_u[0:HS, :], Zi2, mask2,
                op0=mybir.AluOpType.mult, op1=mybir.AluOpType.mult)

            ldw_um = nc.tensor.ldweights(um2_sb[0:HS, 0:1])
            dmy_um = nc.tensor.matmul(dmy_ps[0:1, 0:HS], um2_sb[0:HS, 0:1],
                                      idb[0:HS, 0:HS], start=True, stop=True)
            add_dep_helper(dmy_um.ins, ldw_um.ins, reason="absorb-chain")
            psum_umt = pp.tile([128, NCH, 4, H], bf16, tag="tp")
            for m in range(NCH):
                tp_i = nc.tensor.transpose(
                    psum_umt[:, m, :, :],
                    um2_sb[0:HS, m * 128:(m + 1) * 128], idb[0:HS, 0:HS])
                add_dep_helper(tp_i.ins, dmy_um.ins, reason="absorb")
            umT_sb = mp.tile([128, NCH, 4, H], bf16, tag="umT")
            nc.vector.tensor_copy(umT_sb, psum_umt)
            uT_f = mp.tile([128, NCH, 4], f32, tag="uTf")
            nc.vector.reduce_sum(uT_f, umT_sb, axis=AX.X)
            uT_sb = mp.tile([128, NCH, 4], bf16, tag="uT")
            nc.vector.memset(uT_sb, 0.0)
            for b in range(BL):
                nc.vector.tensor_add(uT_sb[:, :, 2 * b], uT_f[:, :, 2 * b], bvT)

            psum_o = pp.tile([HS, HID], f32, tag="w")
            ldw_ut = nc.tensor.ldweights(uT_sb[:, 0, 0:1])
            ldw_wo = nc.tensor.ldweights(Wo_sb[:, 0, 0:1])
            add_dep_helper(ldw_wo.ins, ldw_ut.ins, reason="absorb-chain")
            first_o = [True]
            for m in range(NCH):
                for hf in range(2):
                    mm = nc.tensor.matmul(
                        psum_o[0:4, hf * 512:(hf + 1) * 512],
                        uT_sb[:, m, :],
                        Wo_sb[:, m, hf * 512:(hf + 1) * 512],
                        start=(m == 0), stop=False)
                    if first_o[0]:
                        add_dep_helper(mm.ins, ldw_wo.ins, reason="absorb")
                        first_o[0] = False
            for hf in range(2):
                nc.tensor.matmul(
                    psum_o[0:4, hf * 512:(hf + 1) * 512],
                    one14, boB[0:1, hf * 512:(hf + 1) * 512],
                    start=False, stop=(hf == 1))
            out_sb = mp.tile([4, HID], f32, tag="osb")
            nc.scalar.copy(out_sb, psum_o[0:4, :])
            prev_pad = None
            for x in range(4):
                p_i = nc.scalar.copy(scratch2[0:1, 6:7], out_sb[0:1, x:x + 1])
                if prev_pad is not None:
                    add_dep_helper(p_i.ins, prev_pad.ins, reason="act-pad-chain")
                prev_pad = p_i
            for b in range(BL):
                nc.scalar.dma_start(out=out_d[b:b + 1, :],
                                    in_=out_sb[2 * b:2 * b + 1, :])

    return nc


def make_in_maps(q, k, v, Wq, bq, Wv, bv, Wo, bo, Wk):
    scale = DH ** -0.5
    mask = np.zeros((H, HID), dtype=np.float32)
    for h in range(H):
        mask[h, h * DH:(h + 1) * DH] = 1.0
    zrow = np.zeros((32 - H, HID), dtype=np.float32)
    mask2 = np.concatenate([mask, zrow, mask, zrow], axis=0)
    shared = {
        "Wq": np.ascontiguousarray(Wq).astype(bf16_np),
        "WkT": np.ascontiguousarray(Wk.T).astype(bf16_np),
        "Wv": np.ascontiguousarray(Wv).astype(bf16_np),
        "Wo": np.ascontiguousarray(Wo).astype(bf16_np),
        "bqB": np.ascontiguousarray((bq * scale).reshape(1, HID)).astype(bf16_np),
        "bvT": np.ascontiguousarray(bv.reshape(NCH, 128).T),
        "boB": np.ascontiguousarray(bo.reshape(1, HID)).astype(bf16_np),
        "one12": np.ones((1, BL), np.float32).astype(bf16_np),
        "one14": np.array([[1.0, 0.0, 1.0, 0.0]], np.float32).astype(bf16_np),
        "identb": np.eye(128, dtype=np.float32).astype(bf16_np),
        "mask2": mask2.astype(bf16_np),
    }
    in_maps = []
    for c in range(NCORES):
        sl = slice(BL * c, BL * (c + 1))
        in_maps.append({
            "kT_loc": np.ascontiguousarray(k[sl].transpose(0, 2, 1)).astype(bf16_np),
            "v_loc": np.ascontiguousarray(v[sl]).astype(bf16_np),
            "qT": np.ascontiguousarray((q[sl] * scale).T).astype(bf16_np),
            **shared,
        })
    return in_maps


_nc_cache = None


def kernel(q, k, v, index_sample, Wq, bq, Wk, bk, Wv, bv, Wo, bo):
    global _nc_cache
    q, k, v = np.asarray(q, np.float32), np.asarray(k, np.float32), np.asarray(v, np.float32)
    Wq, bq = np.asarray(Wq, np.float32), np.asarray(bq, np.float32)
    Wk = np.asarray(Wk, np.float32)
    Wv, bv = np.asarray(Wv, np.float32), np.asarray(bv, np.float32)
    Wo, bo = np.asarray(Wo, np.float32), np.asarray(bo, np.float32)
    # bk provably cancels in the softmax (constant shift per (b, h) row).

    if _nc_cache is None:
        _nc_cache = build_nc()
    nc = _nc_cache
    in_maps = make_in_maps(q, k, v, Wq, bq, Wv, bv, Wo, bo, Wk)
    res = bass_utils.run_bass_kernel_spmd(nc, in_maps, core_ids=list(range(NCORES)))
    out = np.concatenate([r["out_loc"] for r in res.results], axis=0)
    return np.ascontiguousarray(out.astype(np.float32))


# revision 16
# speedup vs baseline: 3.2536x; 1.1358x over previous
"""Trainium2 Bass kernel for ProbSparse multi-head attention (L_Q = 1).

Math: with L_Q=1 the reference's top-k/sampling machinery is identity
(top-1 of a length-1 axis is index 0 and the scatter overwrites the whole
context), so the computation reduces to single-query attention:

  out[b] = concat_h( softmax((q Wq)_h . (k Wk)_h^T / 8) @ (v Wv)_h ) @ Wo + bo

Key algebraic restructuring (L_Q = 1 => low rank):
  scores[b,h,s] = k[b,s,:] . r[b,h,:]      with r[b,h] = Wk_h @ (qh[b,h]/8)
  w[b,h,:]      = sum_s attn[b,h,s] v[b,s,:]
  out[b]        = rowsum_h(masked((w/Z) Wv)) @ Wo + bo

so the big k/v tensors are consumed by exactly one streaming pass each and
never projected through the weight matrices (64x fewer FLOPs).

v2: k arrives pre-transposed (kT, hidden-major) and pre-cast to bf16 from
the host, v pre-cast to bf16, weights pre-cast to bf16.  This removes all
device-side f32->bf16 casts and all PE transposes of k (the v1 bottleneck:
~500us of PE occupancy and 84MB of DMA drop to ~190us PE / 38MB DMA).
The tail (w -> u -> out) is batched over both local batches so Wv / Wo
stream through the PE once per core instead of once per batch.

Sharding: data-parallel over batch, 2 batches per core, 8 cores.

Sync-wait limits (some instruction encodings accept a single semaphore
wait) are handled as in v1: PE transposes are preceded by an
absorber ldweights+matmul pair that soaks up cross-engine waits, and DVE
touch-copies of DMA'd constants early in the program teach the DVE clock
about those DMA ticks before scalar_tensor_tensor needs them.
"""

import sys

sys.path.insert(0, "/opt/trn_rl_repo")

import numpy as np
import ml_dtypes

import concourse.bass as bass
import concourse.mybir as mybir
from bass_rust import add_dep_helper
import concourse.tile_sem_assignment as _tsa
from concourse.tile import TileContext
from concourse import bass_utils


# ---- framework patch: the kernel-tail drain aggregates one semaphore wait
# per active proc, exceeding the 1-wait DRAIN encoding. Split the waits
# across a chain of single-wait drains.
from concourse.tile import TileContext as _TC
from concourse.vector_clock import ScopedClock as _SC

def _split_drain_and_barrier(self, tick_clock, wait_clock):
    drain_inst = self.nc.sync.drain()
    wait_clock.add_sem_waits(drain_inst.ins, _SC({None: tick_clock.global_clock}))
    si = drain_inst.ins.sync_info
    if si is not None and si.on_wait and len(si.on_wait) > 1:
        waits = list(si.on_wait)
        si.on_wait = waits[:1]
        for w in waits[1:]:
            d2 = self.nc.sync.drain()
            s2 = d2.ins.sync_info
            if s2 is None:
                d2.ins.sync_info = type(si)(on_wait=[w], on_update=[])
            else:
                s2.on_wait = [w]
    self.nc.all_engine_barrier()
    assert self.sems is not None
    popped = self.nc._tile_sem_poison_stack.pop()
    assert popped is self._sem_poison
    self.nc.clear_and_free_semaphores(list(self.sems.allocated().values()))
    self.nc.all_engine_barrier()

_TC._drain_and_barrier = _split_drain_and_barrier

B, H, DH, HID, LK = 16, 16, 64, 1024, 4096
NCORES = 8
BL = B // NCORES            # batches per core
NCH = HID // 128            # 8 hidden chunks
NT = LK // 1024             # 4 stream tiles of 1024 seq positions
H2 = BL * H                 # packed (b,h) rows for the r projection
HS = 64                     # tail stacking: batch b at partitions b*32

f32 = mybir.dt.float32
bf16 = mybir.dt.bfloat16
FT = mybir.ActivationFunctionType
AX = mybir.AxisListType

bf16_np = ml_dtypes.bfloat16


def build_nc():
    # one DMA-completion semaphore lane per DGE type: consumers then never
    # accumulate multi-lane DMA waits (several instruction structs allow
    # only 1-2 sync waits).
    _tsa.NUM_HWDGE_SEMS = 1
    _tsa.NUM_SWDGE_GLOBAL_SEMS = 1

    nc = bass.Bass("TRN2")

    kT_d = nc.dram_tensor("kT_loc", [BL, HID, LK], bf16, kind="ExternalInput")
    v_d = nc.dram_tensor("v_loc", [BL, LK, HID], bf16, kind="ExternalInput")
    Wq_d = nc.dram_tensor("Wq", [HID, HID], bf16, kind="ExternalInput")
    WkT_d = nc.dram_tensor("WkT", [HID, HID], bf16, kind="ExternalInput")
    Wv_d = nc.dram_tensor("Wv", [HID, HID], bf16, kind="ExternalInput")
    Wo_d = nc.dram_tensor("Wo", [HID, HID], bf16, kind="ExternalInput")
    blob_d = nc.dram_tensor("blob", [128, 3230], bf16, kind="ExternalInput")
    out_d = nc.dram_tensor("out_loc", [BL, HID], f32, kind="ExternalOutput")

    with TileContext(nc) as tc:
        with tc.tile_pool(name="main", bufs=1) as mp, \
             tc.tile_pool(name="ps", bufs=1, space="PSUM") as pp:

            # ---- constants: one packed DMA (the FIFO chain makes many
            # small const DMAs cost ~1.5us each in serialized latency) ----
            blob = mp.tile([128, 3230], bf16, tag="blob")
            nc.scalar.dma_start(out=blob, in_=blob_d[:, :])
            idb = blob[:, 0:128]
            mask2 = blob[0:HS, 128:128 + HID]
            boB = blob[0:1, 1152:1152 + HID]
            one14 = blob[0:1, 3202:3206]
            bvT = blob[:, 3206:3206 + NCH]

            # DVE touch-copies: teach the DVE clock the const-DMA ticks so
            # later 1-wait DVE structs (scalar_tensor_tensor) don't need a
            # DMA wait slot of their own.
            scratch = mp.tile([128, 8], f32, tag="scratch")
            nc.vector.tensor_copy(scratch[0:HS, 0:1], mask2[:, 0:1])
            nc.vector.tensor_copy(scratch[:, 1:2], bvT[:, 0:1])
            scratch2 = mp.tile([1, 8], f32, tag="scratch2")
            rT_sb = mp.tile([128, NCH, H2], bf16, tag="rT")

            # dedicated never-read psum tile: every absorber dummy writes
            # here, so each dummy carries only the PE WAW-drain wait of its
            # predecessor (a chain) and no DVE WAR.
            dmy_ps = pp.tile([1, 64], f32, tag="dmy")

            # ---- setup: qh = Wq^T qT (+bq), r = WkT^T Qt ----
            # setup-only SBUF lives in a nested pool so its 37KB/partition is
            # reused by the stream tiles afterwards.
            sp_ctx = tc.tile_pool(name="setup", bufs=1)
            sp = sp_ctx.__enter__()
            qT_sb = blob[:, 3214:3214 + NCH * BL].rearrange(
                "p (ch b) -> p ch b", b=BL)
            Wq_sb = sp.tile([128, NCH, HID], bf16, tag="Wq")
            nc.scalar.dma_start(
                out=Wq_sb, in_=Wq_d[:, :].rearrange("(ch p) h -> p ch h", p=128))
            WkT_sb = sp.tile([128, NCH, HID], bf16, tag="WkT")
            nc.scalar.dma_start(
                out=WkT_sb, in_=WkT_d[:, :].rearrange("(ch p) h -> p ch h", p=128))
            bqB = blob[0:1, 2176:2176 + HID]
            one12 = blob[0:1, 3200:3200 + BL]
            # qh[b, :]: lhsT = qT chunk [128, BL], moving = Wq chunk.
            psum_qh = pp.tile([32, HID], f32, tag="w")
            for ch in range(NCH):
                for hf in range(2):
                    nc.tensor.matmul(
                        psum_qh[0:BL, hf * 512:(hf + 1) * 512],
                        qT_sb[:, ch, :],
                        Wq_sb[:, ch, hf * 512:(hf + 1) * 512],
                        start=(ch == 0), stop=False)
            for hf in range(2):
                nc.tensor.matmul(
                    psum_qh[0:BL, hf * 512:(hf + 1) * 512],
                    one12, bqB[0:1, hf * 512:(hf + 1) * 512],
                    start=False, stop=(hf == 1))
            qh_sb = sp.tile([BL, HID], bf16, tag="qh")
            nc.vector.tensor_copy(qh_sb, psum_qh[0:BL, :])

            # transpose qh -> qhT [hd, b] (PE, absorber pattern)
            ldw_q = nc.tensor.ldweights(qh_sb[0:BL, 0:1])
            dmy_q = nc.tensor.matmul(dmy_ps[0:1, 0:BL], qh_sb[0:BL, 0:1],
                                     idb[0:BL, 0:BL], start=True, stop=True)
            add_dep_helper(dmy_q.ins, ldw_q.ins, reason="absorb-chain")
            psum_qt = pp.tile([128, NCH, BL], bf16, tag="tp")
            for ch in range(NCH):
                tp_i = nc.tensor.transpose(
                    psum_qt[:, ch, :],
                    qh_sb[0:BL, ch * 128:(ch + 1) * 128], idb[0:BL, 0:BL])
                add_dep_helper(tp_i.ins, dmy_q.ins, reason="absorb")
            qhT_sb = sp.tile([128, NCH, BL], bf16, tag="qhT")
            nc.vector.tensor_copy(qhT_sb, psum_qt)

            # Qt: block-diag expansion [hd, (ch, b, h)], h == head(hd)
            Qt_sb = sp.tile([128, NCH, BL, H], bf16, tag="Qt")
            nc.vector.memset(Qt_sb, 0.0)
            for m in range(NCH):
                for g in range(2):
                    h = 2 * m + g
                    nc.vector.tensor_copy(
                        Qt_sb[g * 64:(g + 1) * 64, m, :, h],
                        qhT_sb[g * 64:(g + 1) * 64, m, :])

            # rTT[(b h), c] = sum_hd Qt[hd, (b h)] WkT[hd, c]
            psum_rTT = pp.tile([32, HID], f32, tag="w")
            ldw_wk = nc.tensor.ldweights(WkT_sb[:, 0, 0:1])
            first_rtt = [True]
            for ch in range(NCH):
                for hf in range(2):
                    mm = nc.tensor.matmul(
                        psum_rTT[0:H2, hf * 512:(hf + 1) * 512],
                        Qt_sb[:, ch, :, :],
                        WkT_sb[:, ch, hf * 512:(hf + 1) * 512],
                        start=(ch == 0), stop=(ch == NCH - 1))
                    if first_rtt[0]:
                        add_dep_helper(mm.ins, ldw_wk.ins, reason="absorb")
                        first_rtt[0] = False
            rTT_sb = sp.tile([H2, HID], bf16, tag="rTT")
            nc.vector.tensor_copy(rTT_sb, psum_rTT[0:H2, :])

            # transpose rTT -> rT [c, (b h)]
            ldw_r = nc.tensor.ldweights(rTT_sb[0:H2, 0:1])
            dmy_r = nc.tensor.matmul(dmy_ps[0:1, 0:H2], rTT_sb[0:H2, 0:1],
                                     idb[0:H2, 0:H2], start=True, stop=True)
            add_dep_helper(dmy_r.ins, ldw_r.ins, reason="absorb-chain")
            psum_rt = pp.tile([128, NCH, H2], bf16, tag="tp")
            for cj in range(NCH):
                tp_i = nc.tensor.transpose(
                    psum_rt[:, cj, :],
                    rTT_sb[0:H2, cj * 128:(cj + 1) * 128], idb[0:H2, 0:H2])
                add_dep_helper(tp_i.ins, dmy_r.ins, reason="absorb")
            nc.vector.tensor_copy(rT_sb, psum_rt)
            sp_ctx.__exit__(None, None, None)
            # PE marker into the dmy_ps corner, then an ACT touch of that
            # corner: the touch carries one PE RAW wait and ratchets the ACT
            # clock over all setup PE work.
            nc.tensor.matmul(dmy_ps[0:1, 48:49], rTT_sb[0:1, 0:1],
                             rTT_sb[0:1, 0:1], start=True, stop=True)
            nc.scalar.copy(scratch2[0:1, 0:1], dmy_ps[0:1, 48:49])

            # ---- streaming batches ----
            psum_w = [None, None]
            Zi2 = mp.tile([HS, 1], f32, tag="Zi2")
            nc.vector.memset(Zi2, 0.0)
            last_psum_at = [None]
            attnT_sb_prev = [None]
            for bl in range(BL):
                if bl > 0:
                    # ratchet ACT past batch bl-1's V phase (the marker) so
                    # this batch's kt/vt triggers carry only their DMA WAW.
                    nc.scalar.copy(scratch2[0:1, 1:2], dmy_ps[0:1, 48:49])
                # scores phase ------------------------------------------------
                scores_sb = mp.tile([H, LK], bf16, tag="scores", bufs=1)
                m8 = mp.tile([H, 2 * NT], f32, tag="m8", bufs=2)
                for t in range(NT):
                    kt = mp.tile([128, NCH, 1024], bf16, tag="kt", bufs=4)
                    nc.scalar.dma_start(
                        out=kt,
                        in_=kT_d[bl, :, t * 1024:(t + 1) * 1024]
                        .rearrange("(ch p) s -> p ch s", p=128))
                    ldw_k = nc.tensor.ldweights(kt[:, 0, 0:1])
                    prev_mm = None
                    for sh in range(2):
                        blk = t * 2 + sh
                        psum_s = pp.tile([33, 512], f32, tag="s")
                        dmy = nc.tensor.matmul(
                            dmy_ps[0:1, 0:1], kt[:, 0, 0:1], kt[:, 0, 0:1],
                            start=True, stop=True)
                        add_dep_helper(dmy.ins, (ldw_k if sh == 0 else prev_mm).ins,
                                       reason="absorb-chain")
                        for cj in range(NCH):
                            mm = nc.tensor.matmul(
                                psum_s[0:H, :],
                                rT_sb[:, cj, bl * H:(bl + 1) * H],
                                kt[:, cj, sh * 512:(sh + 1) * 512],
                                start=(cj == 0), stop=(cj == NCH - 1))
                            if cj == 0:
                                add_dep_helper(mm.ins, dmy.ins, reason="absorb")
                            prev_mm = mm
                        nc.vector.reduce_max(m8[:, blk:blk + 1], psum_s[0:H, :],
                                             axis=AX.X)
                        nc.vector.tensor_copy(
                            scores_sb[:, blk * 512:(blk + 1) * 512], psum_s[0:H, :])

                # scores-complete marker + ACT ratchet: placed a whole phase
                # upstream of the next batch's kt triggers so the scheduler's
                # DMA hoisting cannot lift them above it.
                nc.tensor.matmul(dmy_ps[0:1, 48:49], kt[0:1, 0, 0:1],
                                 kt[0:1, 0, 0:1], start=True, stop=True)
                nc.scalar.copy(scratch2[0:1, 4 + bl:5 + bl],
                               dmy_ps[0:1, 48:49])

                # softmax -----------------------------------------------------
                negmax = mp.tile([H, 1], f32, tag="negmax", bufs=2)
                nc.vector.reduce_max(negmax, m8, axis=AX.X, negate=True)
                attn_sb = mp.tile([H, LK], bf16, tag="attn", bufs=2)
                Zs = mp.tile([H, 2 * NT], f32, tag="Zs", bufs=2)
                for jj in range(2 * NT):
                    nc.scalar.activation(
                        attn_sb[:, jj * 512:(jj + 1) * 512],
                        scores_sb[:, jj * 512:(jj + 1) * 512],
                        FT.Exp, bias=negmax, scale=1.0,
                        accum_out=Zs[:, jj:jj + 1])
                Z = mp.tile([H, 1], f32, tag="Z", bufs=2)
                nc.vector.reduce_sum(Z, Zs, axis=AX.X)
                nc.vector.reciprocal(Zi2[bl * 32:bl * 32 + H, :], Z)

                # attn transposed on PE -> attnT [s, h]
                ldw_at = nc.tensor.ldweights(attn_sb[0:H, 0:1])
                dmy_at = nc.tensor.matmul(dmy_ps[0:1, 0:H], attn_sb[0:H, 0:1],
                                          idb[0:H, 0:H], start=True, stop=True)
                add_dep_helper(dmy_at.ins, ldw_at.ins, reason="absorb-chain")
                attnT_sb = mp.tile([128, LK // 128, H], bf16, tag="attnT", bufs=1)
                for g in range(2):
                    psum_at = pp.tile([128, 16, H], bf16, tag="tp")
                    for tt in range(16):
                        j = g * 16 + tt
                        tp_i = nc.tensor.transpose(
                            psum_at[:, tt, :],
                            attn_sb[0:H, j * 128:(j + 1) * 128], idb[0:H, 0:H])
                        add_dep_helper(tp_i.ins, dmy_at.ins, reason="absorb")
                    nc.vector.tensor_copy(
                        attnT_sb[:, g * 16:(g + 1) * 16, :], psum_at)
                    last_psum_at[0] = psum_at
                attnT_sb_prev[0] = attnT_sb

                # weighted-V phase -------------------------------------------
                pw = pp.tile([32, HID], f32, tag="w")
                psum_w[bl] = pw
                for t in range(NT):
                    vt = mp.tile([128, NCH, HID], bf16, tag="vt", bufs=4)
                    nc.scalar.dma_start(
                        out=vt,
                        in_=v_d[bl, t * 1024:(t + 1) * 1024, :]
                        .rearrange("(blk p) c -> p blk c", p=128))
                    ldw_v = nc.tensor.ldweights(vt[:, 0, 0:1])
                    dmy_v = nc.tensor.matmul(
                        dmy_ps[0:1, 0:1], vt[:, 0, 0:1], vt[:, 0, 0:1],
                        start=True, stop=True)
                    add_dep_helper(dmy_v.ins, ldw_v.ins, reason="absorb-chain")
                    for blk in range(NCH):
                        for hf in range(2):
                            mm = nc.tensor.matmul(
                                pw[0:H, hf * 512:(hf + 1) * 512],
                                attnT_sb[:, t * NCH + blk, :],
                                vt[:, blk, hf * 512:(hf + 1) * 512],
                                start=(t == 0 and blk == 0),
                                stop=(t == NT - 1 and blk == NCH - 1))
                            if blk == 0 and hf == 0:
                                add_dep_helper(mm.ins, dmy_v.ins, reason="absorb")
                # V-phase-complete marker for the next batch's ACT ratchet
                nc.tensor.matmul(dmy_ps[0:1, 48:49], vt[0:1, 0, 0:1],
                                 vt[0:1, 0, 0:1], start=True, stop=True)

            # ---- batched tail: w -> u -> out for both batches at once ----
            w2_sb = mp.tile([HS, HID], bf16, tag="w2")
            nc.vector.memset(w2_sb, 0.0)
            for bl in range(BL):
                nc.vector.tensor_copy(w2_sb[bl * 32:bl * 32 + H, :],
                                      psum_w[bl][0:H, :])
            # ratchet ACT past batch 1's V accumulation, then pull the tail
            # weights into recycled stream-tile slots.
            nc.scalar.copy(scratch2[0:1, 3:4], dmy_ps[0:1, 48:49])
            Wv_sb = mp.tile([128, NCH, HID], bf16, tag="kt", bufs=4)
            nc.scalar.dma_start(
                out=Wv_sb, in_=Wv_d[:, :].rearrange("(ch p) h -> p ch h", p=128))
            Wo_sb = mp.tile([128, NCH, HID], bf16, tag="vt", bufs=4)
            nc.scalar.dma_start(
                out=Wo_sb, in_=Wo_d[:, :].rearrange("(ch p) h -> p ch h", p=128))
            ldw_w = nc.tensor.ldweights(w2_sb[0:HS, 0:1])
            dmy_w = nc.tensor.matmul(dmy_ps[0:1, 0:HS], w2_sb[0:HS, 0:1],
                                     idb[0:HS, 0:HS], start=True, stop=True)
            add_dep_helper(dmy_w.ins, ldw_w.ins, reason="absorb-chain")
            psum_wt = pp.tile([128, NCH, HS], bf16, tag="tp")
            for cj in range(NCH):
                tp_i = nc.tensor.transpose(
                    psum_wt[:, cj, :],
                    w2_sb[0:HS, cj * 128:(cj + 1) * 128], idb[0:HS, 0:HS])
                add_dep_helper(tp_i.ins, dmy_w.ins, reason="absorb")
            wT_sb = mp.tile([128, NCH, HS], bf16, tag="wT")
            nc.vector.tensor_copy(wT_sb, psum_wt)

            psum_u = pp.tile([HS, HID], f32, tag="w")
            ldw_wv = nc.tensor.ldweights(Wv_sb[:, 0, 0:1])
            dmy_u = nc.tensor.matmul(dmy_ps[0:1, 0:1], wT_sb[:, 0, 0:1],
                                     wT_sb[:, 0, 0:1], start=True, stop=True)
            add_dep_helper(dmy_u.ins, ldw_wv.ins, reason="absorb-chain")
            first_u = [True]
            for cj in range(NCH):
                for hf in range(2):
                    mm = nc.tensor.matmul(
                        psum_u[0:HS, hf * 512:(hf + 1) * 512],
                        wT_sb[:, cj, :],
                        Wv_sb[:, cj, hf * 512:(hf + 1) * 512],
                        start=(cj == 0), stop=(cj == NCH - 1))
                    if first_u[0]:
                        add_dep_helper(mm.ins, dmy_u.ins, reason="absorb")
                        first_u[0] = False
            um2_sb = mp.tile([HS, HID], bf16, tag="um2")
            nc.vector.tensor_copy(scratch[0:1, 3:4], psum_u[0:1, 0:1])
            nc.vector.scalar_tensor_tensor(
                um2_sb, psum_u[0:HS, :], Zi2, mask2,
                op0=mybir.AluOpType.mult, op1=mybir.AluOpType.mult)

            ldw_um = nc.tensor.ldweights(um2_sb[0:HS, 0:1])
            dmy_um = nc.tensor.matmul(dmy_ps[0:1, 0:HS], um2_sb[0:HS, 0:1],
                                      idb[0:HS, 0:HS], start=True, stop=True)
            add_dep_helper(dmy_um.ins, ldw_um.ins, reason="absorb-chain")
            psum_umt = pp.tile([128, NCH, 4, H], bf16, tag="tp")
            for m in range(NCH):
                tp_i = nc.tensor.transpose(
                    psum_umt[:, m, :, :],
                    um2_sb[0:HS, m * 128:(m + 1) * 128], idb[0:HS, 0:HS])
                add_dep_helper(tp_i.ins, dmy_um.ins, reason="absorb")
            umT_sb = mp.tile([128, NCH, 4, H], bf16, tag="umT")
            nc.vector.tensor_copy(umT_sb, psum_umt)
            uT_f = mp.tile([128, NCH, 4], f32, tag="uTf")
            nc.vector.reduce_sum(uT_f, umT_sb, axis=AX.X)
            uT_sb = mp.tile([128, NCH, 4], bf16, tag="uT")
            nc.vector.memset(uT_sb, 0.0)
            for b in range(BL):
                nc.vector.tensor_add(uT_sb[:, :, 2 * b], uT_f[:, :, 2 * b], bvT)

            psum_o = pp.tile([HS, HID], f32, tag="w")
            ldw_ut = nc.tensor.ldweights(uT_sb[:, 0, 0:1])
            ldw_wo = nc.tensor.ldweights(Wo_sb[:, 0, 0:1])
            add_dep_helper(ldw_wo.ins, ldw_ut.ins, reason="absorb-chain")
            first_o = [True]
            for m in range(NCH):
                for hf in range(2):
                    mm = nc.tensor.matmul(
                        psum_o[0:4, hf * 512:(hf + 1) * 512],
                        uT_sb[:, m, :],
                        Wo_sb[:, m, hf * 512:(hf + 1) * 512],
                        start=(m == 0), stop=False)
                    if first_o[0]:
                        add_dep_helper(mm.ins, ldw_wo.ins, reason="absorb")
                        first_o[0] = False
            for hf in range(2):
                nc.tensor.matmul(
                    psum_o[0:4, hf * 512:(hf + 1) * 512],
                    one14, boB[0:1, hf * 512:(hf + 1) * 512],
                    start=False, stop=(hf == 1))
            out_sb = mp.tile([4, HID], f32, tag="osb")
            nc.scalar.copy(out_sb, psum_o[0:4, :])
            prev_pad = None
            for x in range(4):
                p_i = nc.scalar.copy(scratch2[0:1, 6:7], out_sb[0:1, x:x + 1])
                if prev_pad is not None:
                    add_dep_helper(p_i.ins, prev_pad.ins, reason="act-pad-chain")
                prev_pad = p_i
            for b in range(BL):
                nc.scalar.dma_start(out=out_d[b:b + 1, :],
                                    in_=out_sb[2 * b:2 * b + 1, :])

    return nc


def make_in_maps(q, k, v, Wq, bq, Wv, bv, Wo, bo, Wk):
    scale = DH ** -0.5
    mask = np.zeros((H, HID), dtype=np.float32)
    for h in range(H):
        mask[h, h * DH:(h + 1) * DH] = 1.0
    zrow = np.zeros((32 - H, HID), dtype=np.float32)
    mask2 = np.concatenate([mask, zrow, mask, zrow], axis=0)
    blob = np.zeros((128, 3230), dtype=np.float32)
    blob[:, 0:128] = np.eye(128, dtype=np.float32)
    blob[0:HS, 128:128 + HID] = mask2
    blob[0:1, 1152:1152 + HID] = bo.reshape(1, HID)
    blob[0:1, 2176:2176 + HID] = (bq * scale).reshape(1, HID)
    blob[0:1, 3200:3200 + BL] = 1.0
    blob[0:1, 3202:3206] = np.array([1.0, 0.0, 1.0, 0.0])
    blob[:, 3206:3206 + NCH] = bv.reshape(NCH, 128).T
    shared = {
        "Wq": np.ascontiguousarray(Wq).astype(bf16_np),
        "WkT": np.ascontiguousarray(Wk.T).astype(bf16_np),
        "Wv": np.ascontiguousarray(Wv).astype(bf16_np),
        "Wo": np.ascontiguousarray(Wo).astype(bf16_np),
    }
    in_maps = []
    for c in range(NCORES):
        sl = slice(BL * c, BL * (c + 1))
        bc = blob.copy()
        qT = (q[sl] * scale).T  # [HID, BL]
        bc[:, 3214:3214 + NCH * BL] = qT.reshape(NCH, 128, BL).transpose(
            1, 0, 2).reshape(128, NCH * BL)
        in_maps.append({
            "kT_loc": np.ascontiguousarray(k[sl].transpose(0, 2, 1)).astype(bf16_np),
            "v_loc": np.ascontiguousarray(v[sl]).astype(bf16_np),
            "blob": bc.astype(bf16_np),
            **shared,
        })
    return in_maps


_nc_cache = None


def kernel(q, k, v, index_sample, Wq, bq, Wk, bk, Wv, bv, Wo, bo):
    global _nc_cache
    q, k, v = np.asarray(q, np.float32), np.asarray(k, np.float32), np.asarray(v, np.float32)
    Wq, bq = np.asarray(Wq, np.float32), np.asarray(bq, np.float32)
    Wk = np.asarray(Wk, np.float32)
    Wv, bv = np.asarray(Wv, np.float32), np.asarray(bv, np.float32)
    Wo, bo = np.asarray(Wo, np.float32), np.asarray(bo, np.float32)
    # bk provably cancels in the softmax (constant shift per (b, h) row).

    if _nc_cache is None:
        _nc_cache = build_nc()
    nc = _nc_cache
    in_maps = make_in_maps(q, k, v, Wq, bq, Wv, bv, Wo, bo, Wk)
    res = bass_utils.run_bass_kernel_spmd(nc, in_maps, core_ids=list(range(NCORES)))
    out = np.concatenate([r["out_loc"] for r in res.results], axis=0)
    return np.ascontiguousarray(out.astype(np.float32))
